# revision 10
# baseline (speedup 1.0000x reference)
"""Trainium2 Bass kernel for nn_Deep_AD_F_58213986730479 (dense_cnn).

Math (per iteration t of 3):
    feats = 4 one-pixel zero-padded shifts (N,S,W,E) of x        [n,4,h,w]
    d     = conv3x3(feats, W[t]) + b[t]                          [n,4,h,w]
    x    -= sum_k d_k * exp(-d_k^2) / 4

Implementation:
  - Pure data parallel: batch 32 -> 8 cores x 4 images.
  - The shift+conv composes into a 21-tap stencil on x. Vertical taps are
    applied with banded-matrix matmuls on TensorE (contraction over image
    rows on partitions); horizontal taps via 5 column-shifted accumulating
    matmuls into PSUM. Boundary semantics of the double zero-padding are
    exact: row-edge terms fold into per-tile band-matrix variants; column
    edge terms are two N=1 correction matmuls per channel.
  - exp(-d^2) comes from one ScalarE op: Derivative_Erf = 2/sqrt(pi)*exp(-x^2);
    the bias add (d+b) rides free in the activation and in the DVE
    scalar_tensor_tensor that forms gated = (d+b)*e. Channel sum on GpSimd,
    final x update is one fused DVE scalar_tensor_tensor.
  - Each 512x512 image is 5 row-tiles [128,512] (stride 116, 6-row halo);
    3 iterations shrink the valid halo by 2 rows each, so no cross-tile
    traffic is ever needed.
"""
import sys

sys.path.insert(0, "/opt/trn_rl_repo")

import math
import numpy as np

import concourse.bass as bass
import concourse.bacc as bacc
import concourse.mybir as mybir
from concourse.tile import TileContext
from concourse.bass_utils import run_bass_kernel_spmd

F32 = mybir.dt.float32
AF = mybir.ActivationFunctionType
ALU = mybir.AluOpType

NCORES = 8
IMGS = 4          # images per core
H = W_IMG = 512
T_ITERS = 3
KCH = 4
NTILES = 5
TSTART = [-6, 110, 226, 342, 458]   # image row held by partition 0 of tile j
CORE_LO = 6                          # first owned partition of each tile
CORE_ROWS = [116, 116, 116, 116, 48]
C_UPD = math.sqrt(math.pi) / 8.0     # 1/4 * sqrt(pi)/2 (Derivative_Erf scale)

# feats channel order in reference: N, S, W, E
OY = [-1, 1, 0, 0]
OX = [0, 0, -1, 1]

DXS = [0, -1, 1, -2, 2]

# debug bisect flags
_SKIP_CORR = False
_PSUM_BUFS = 2
_INPLACE_UPD = True
_MASK_AP = True
_TILE_SET = None  # e.g. [2] to restrict tiles (debug)
_PAD_BMAT = True
_SIMPLE_BIAS = False  # Dx=0 first: full-range start=True write


def _composite_taps(Wc):
    """T[t,k,Dy+2,Dx+2] = sum of W[t,k,i,dy+1,dx+1] with dy+oy_i=Dy, dx+ox_i=Dx."""
    taps = np.zeros((T_ITERS, KCH, 5, 5), np.float64)
    for t in range(T_ITERS):
        for k in range(KCH):
            for i in range(4):
                for dy in (-1, 0, 1):
                    for dx in (-1, 0, 1):
                        taps[t, k, dy + OY[i] + 2, dx + OX[i] + 2] += Wc[
                            t, k, i, dy + 1, dx + 1
                        ]
    return taps


def _build_bmats(Wc):
    """Dense lhsT matrices, returned as array [NB,128,128] f32 plus an index fn.

    Layout per (t,k): 5 generic B_Dx, then 3 top-variant (Dx=-1,0,1), then
    3 bottom-variant, then left corr, right corr = 13 matrices.
    B[in_row, out_row] = tap[in-out, Dx].
    """
    taps = _composite_taps(Wc)
    mats = []
    index = {}

    def band(vals_by_dy):
        B = np.zeros((128, 128), np.float64)
        for dy, v in vals_by_dy.items():
            B += v * np.eye(128, k=-dy)
        return B

    for t in range(T_ITERS):
        for k in range(KCH):
            per_dx = {}
            for Dx in (-2, -1, 0, 1, 2):
                per_dx[Dx] = band(
                    {Dy: taps[t, k, Dy + 2, Dx + 2] for Dy in range(-2, 3)}
                )
            for Dx in DXS:
                index[(t, k, Dx, "mid")] = len(mats)
                mats.append(per_dx[Dx])
            for Dx in (-1, 0, 1):
                Btop = per_dx[Dx].copy()
                # image row 0 = partition CORE_LO of tile 0: remove south-ch dy=-1
                Btop[CORE_LO, CORE_LO] -= Wc[t, k, 1, 0, Dx + 1]
                index[(t, k, Dx, "top")] = len(mats)
                mats.append(Btop)
            for Dx in (-1, 0, 1):
                Bbot = per_dx[Dx].copy()
                # image row 511 = partition 53 of tile 4: remove north-ch dy=+1
                p = CORE_LO + (H - 1) - TSTART[4] - CORE_LO  # = 53
                Bbot[p, p] -= Wc[t, k, 0, 2, Dx + 1]
                index[(t, k, Dx, "bot")] = len(mats)
                mats.append(Bbot)
            # column-edge corrections (vertical 3-tap bands)
            Bl = band({dy: -Wc[t, k, 3, dy + 1, 0] for dy in (-1, 0, 1)})
            index[(t, k, "corrL")] = len(mats)
            mats.append(Bl)
            Br = band({dy: -Wc[t, k, 2, dy + 1, 2] for dy in (-1, 0, 1)})
            index[(t, k, "corrR")] = len(mats)
            mats.append(Br)
    arr = np.stack(mats).astype(np.float32)
    return arr, index


_NB = T_ITERS * KCH * 13


def _build_masks():
    """Per-tile update masks [128, NTILES]: -C_UPD at real image rows, 0 at pad."""
    m = np.full((128, NTILES), -C_UPD, np.float32)
    for j in range(NTILES):
        r0 = TSTART[j]
        plo = max(0, -r0)
        phi = min(128, H - r0)
        m[0:plo, j] = 0.0
        m[phi:128, j] = 0.0
    return m


def _build_nc(bvals, bindex):
    nc = bacc.Bacc(None, target_bir_lowering=False)
    xs = nc.declare_dram_parameter("xs", [IMGS, H, W_IMG], F32, isOutput=False)
    bmw = _NB * 128 + NTILES
    if _PAD_BMAT:
        bmw = (bmw + 127) // 128 * 128
    bm = nc.declare_dram_parameter("bmat", [128, bmw], F32, isOutput=False)
    yo = nc.declare_dram_parameter("out", [IMGS, H, W_IMG], F32, isOutput=True)

    with TileContext(nc) as tc:
        with (
            tc.tile_pool(name="wts", bufs=1) as wp,
            tc.tile_pool(name="xdata", bufs=1) as xp,
            tc.tile_pool(name="work", bufs=2) as sp,
            tc.tile_pool(name="ps", bufs=_PSUM_BUFS, space="PSUM") as pp,
        ):
            bmt = wp.tile([128, bmw], F32, tag="bmt")
            nc.sync.dma_start(out=bmt[:], in_=bm[:])

            if _SIMPLE_BIAS:
                bias_tiles = {}
                for t in range(T_ITERS):
                    for k in range(KCH):
                        bb = wp.tile([128, 1], F32, tag=f"bias{t}_{k}")
                        nc.vector.memset(bb[:], float(bvals[t, k]))
                        bias_tiles[t, k] = bb
            else:
                bias_t = wp.tile([128, T_ITERS * KCH], F32, tag="bias")
                for t in range(T_ITERS):
                    for k in range(KCH):
                        nc.vector.memset(
                            bias_t[:, t * KCH + k : t * KCH + k + 1], float(bvals[t, k])
                        )

            def bmat(idx):
                return bmt[:, idx * 128 : (idx + 1) * 128]

            tset = _TILE_SET if _TILE_SET is not None else list(range(NTILES))
            xt = {}
            for im in range(IMGS):
                for j in tset:
                    tile = xp.tile([128, W_IMG], F32, tag=f"x{im}_{j}")
                    xt[im, j] = tile
                    r0 = TSTART[j]
                    plo = max(0, -r0)
                    phi = min(128, H - r0)
                    if plo > 0 or phi < 128:
                        nc.vector.memset(tile[:], 0.0)
                    nc.sync.dma_start(
                        out=tile[plo:phi, :], in_=xs[im, r0 + plo : r0 + phi, :]
                    )

            for it in range(T_ITERS):
                for im in range(IMGS):
                    for j in tset:
                        x_t = xt[im, j]
                        cls = "top" if j == 0 else ("bot" if j == NTILES - 1 else "mid")
                        d = pp.tile([128, KCH * W_IMG], F32, tag="d")
                        for k in range(KCH):
                            base = k * W_IMG
                            for Dx in DXS:
                                key = (
                                    (it, k, Dx, cls)
                                    if (it, k, Dx, cls) in bindex
                                    else (it, k, Dx, "mid")
                                )
                                ocl = max(0, -Dx)
                                och = W_IMG - max(0, Dx)
                                nc.tensor.matmul(
                                    d[:, base + ocl : base + och],
                                    bmat(bindex[key]),
                                    x_t[:, ocl + Dx : och + Dx],
                                    start=(Dx == 0),
                                    stop=False,
                                )
                            if not _SKIP_CORR:
                                nc.tensor.matmul(
                                    d[:, base : base + 1],
                                    bmat(bindex[(it, k, "corrL")]),
                                    x_t[:, 0:1],
                                    start=False,
                                    stop=False,
                                )
                                nc.tensor.matmul(
                                    d[:, base + W_IMG - 1 : base + W_IMG],
                                    bmat(bindex[(it, k, "corrR")]),
                                    x_t[:, W_IMG - 1 : W_IMG],
                                    start=False,
                                    stop=True,
                                )
                        e = sp.tile([128, KCH * W_IMG], F32, tag="e")
                        for k in range(KCH):
                            base = k * W_IMG
                            nc.scalar.activation(
                                e[:, base : base + W_IMG],
                                d[:, base : base + W_IMG],
                                AF.Derivative_Erf,
                                bias=(bias_tiles[it, k][:, 0:1] if _SIMPLE_BIAS
                                      else bias_t[:, it * KCH + k : it * KCH + k + 1]),
                                scale=1.0,
                            )
                        g = sp.tile([128, KCH * W_IMG], F32, tag="g")
                        for k in range(KCH):
                            base = k * W_IMG
                            nc.vector.scalar_tensor_tensor(
                                out=g[:, base : base + W_IMG],
                                in0=d[:, base : base + W_IMG],
                                scalar=float(bvals[it, k]),
                                in1=e[:, base : base + W_IMG],
                                op0=ALU.add,
                                op1=ALU.mult,
                            )
                        s01 = sp.tile([128, W_IMG], F32, tag="s01")
                        s23 = sp.tile([128, W_IMG], F32, tag="s23")
                        stot = sp.tile([128, W_IMG], F32, tag="stot")
                        nc.gpsimd.tensor_tensor(
                            out=s01[:], in0=g[:, 0:512], in1=g[:, 512:1024], op=ALU.add
                        )
                        nc.gpsimd.tensor_tensor(
                            out=s23[:], in0=g[:, 1024:1536], in1=g[:, 1536:2048],
                            op=ALU.add,
                        )
                        nc.gpsimd.tensor_tensor(
                            out=stot[:], in0=s01[:], in1=s23[:], op=ALU.add
                        )
                        mask_ap = (
                            bmt[:, _NB * 128 + j : _NB * 128 + j + 1]
                            if _MASK_AP
                            else -C_UPD
                        )
                        if _INPLACE_UPD:
                            nc.vector.scalar_tensor_tensor(
                                out=x_t[:],
                                in0=stot[:],
                                scalar=mask_ap,
                                in1=x_t[:],
                                op0=ALU.mult,
                                op1=ALU.add,
                            )
                        else:
                            x_new = xp.tile([128, W_IMG], F32, tag=f"xn{im}_{j}_{it}")
                            nc.vector.scalar_tensor_tensor(
                                out=x_new[:],
                                in0=stot[:],
                                scalar=mask_ap,
                                in1=x_t[:],
                                op0=ALU.mult,
                                op1=ALU.add,
                            )
                            xt[im, j] = x_new

            for im in range(IMGS):
                for j in tset:
                    rows = CORE_ROWS[j]
                    nc.sync.dma_start(
                        out=yo[im, 116 * j : 116 * j + rows, :],
                        in_=xt[im, j][CORE_LO : CORE_LO + rows, :],
                    )
    nc.compile()
    return nc


_CACHE = {}


def _get_program(Wc, bc):
    key = (Wc.tobytes(), bc.tobytes())
    if key not in _CACHE:
        barr, bindex = _build_bmats(Wc.astype(np.float64))
        # SBUF layout [p, n*128+m]
        parts = [barr.transpose(1, 0, 2).reshape(128, _NB * 128), _build_masks()]
        w0 = _NB * 128 + NTILES
        if _PAD_BMAT:
            wpad = (w0 + 127) // 128 * 128 - w0
            parts.append(np.zeros((128, wpad), np.float32))
        bflat = np.ascontiguousarray(np.concatenate(parts, axis=1), dtype=np.float32)
        nc = _build_nc(bc.astype(np.float64), bindex)
        _CACHE[key] = (nc, bflat)
    return _CACHE[key]


def _install_trace_shim():
    """The agent image lacks antenv.axon_hooks; rebuild the NTFF hook from
    trn_boot's ctypes recipe and skip the artifact upload."""
    import types

    if "antenv.axon_hooks" in sys.modules:
        return
    try:
        from trn_agent_boot.trn_boot import _ntff_profile_via_ctypes

        hook = _ntff_profile_via_ctypes("/opt/axon/libaxon_pjrt.so")
    except Exception:
        hook = None
    mod = types.ModuleType("antenv.axon_hooks")
    mod.get_axon_ntff_profile_hook = lambda: hook
    mod.set_axon_ntff_profile_hook = lambda h: None
    sys.modules["antenv.axon_hooks"] = mod
    import concourse.bass_utils as bu

    bu.upload_artifacts = lambda d: "local://skipped"


def kernel(x, W, b, _trace=False, _tracedir=None):
    x = np.asarray(x)
    W = np.asarray(W)
    b = np.asarray(b)
    nc, bflat = _get_program(W, b)
    in_maps = []
    for c in range(NCORES):
        shard = np.ascontiguousarray(x[c * IMGS : (c + 1) * IMGS, 0]).astype(np.float32)
        in_maps.append({"xs": shard, "bmat": bflat})
    kw = {}
    if _trace:
        _install_trace_shim()
        kw = {"trace": True, "tmpdir": _tracedir}
    res = run_bass_kernel_spmd(nc, in_maps, list(range(NCORES)), **kw)
    out = np.concatenate([res.results[c]["out"] for c in range(NCORES)], axis=0)
    out = out[:, None].astype(x.dtype)
    kernel._last = res
    return out


# revision 12
# speedup vs baseline: 2.8337x; 2.8337x over previous
"""Trainium2 Bass kernel for nn_Deep_AD_F_58213986730479 (dense_cnn).

Math (per iteration t of 3):
    feats = 4 one-pixel zero-padded shifts (N,S,W,E) of x        [n,4,h,w]
    d     = conv3x3(feats, W[t]) + b[t]                          [n,4,h,w]
    x    -= sum_k d_k * exp(-d_k^2) / 4

Implementation:
  - Pure data parallel: batch 32 -> 8 cores x 4 images.
  - The shift+conv composes into a 21-tap stencil on x. Vertical taps are
    applied with banded-matrix matmuls on TensorE (contraction over image
    rows on partitions); horizontal taps via 5 column-shifted accumulating
    matmuls into PSUM. Boundary semantics of the double zero-padding are
    exact: row-edge terms fold into per-tile band-matrix variants; column
    edge terms are two N=1 correction matmuls per channel.
  - exp(-d^2) comes from one ScalarE op: Derivative_Erf = 2/sqrt(pi)*exp(-x^2);
    the bias add (d+b) rides free in the activation and in the DVE
    scalar_tensor_tensor that forms gated = (d+b)*e. Channel sum on GpSimd,
    final x update is one fused DVE scalar_tensor_tensor.
  - Each 512x512 image is 5 row-tiles [128,512] (stride 116, 6-row halo);
    3 iterations shrink the valid halo by 2 rows each, so no cross-tile
    traffic is ever needed.
"""
import sys

sys.path.insert(0, "/opt/trn_rl_repo")

import math
import numpy as np

import concourse.bass as bass
import concourse.bacc as bacc
import concourse.mybir as mybir
from concourse.tile import TileContext
from concourse.bass_utils import run_bass_kernel_spmd

F32 = mybir.dt.float32
F32R = mybir.dt.float32r
BF16 = mybir.dt.bfloat16
AF = mybir.ActivationFunctionType
ALU = mybir.AluOpType

NCORES = 8
IMGS = 4          # images per core
H = W_IMG = 512
T_ITERS = 3
KCH = 4
NTILES = 5
TSTART = [-6, 110, 226, 342, 458]   # image row held by partition 0 of tile j
CORE_LO = 6                          # first owned partition of each tile
CORE_ROWS = [116, 116, 116, 116, 48]
C_UPD = math.sqrt(math.pi) / 8.0     # 1/4 * sqrt(pi)/2 (Derivative_Erf scale)

# feats channel order in reference: N, S, W, E
OY = [-1, 1, 0, 0]
OX = [0, 0, -1, 1]

DXS = [0, -1, 1, -2, 2]

# debug bisect flags
_SKIP_CORR = False
_PSUM_BUFS = 2
_INPLACE_UPD = True
_MASK_AP = True
_TILE_SET = None  # e.g. [2] to restrict tiles (debug)
_PAD_BMAT = True
_MM_DTYPE = __import__("os").environ.get("KERNEL_MM_DTYPE", "f32")  # f32 | f32r | bf16
_SIMPLE_BIAS = False  # Dx=0 first: full-range start=True write


def _composite_taps(Wc):
    """T[t,k,Dy+2,Dx+2] = sum of W[t,k,i,dy+1,dx+1] with dy+oy_i=Dy, dx+ox_i=Dx."""
    taps = np.zeros((T_ITERS, KCH, 5, 5), np.float64)
    for t in range(T_ITERS):
        for k in range(KCH):
            for i in range(4):
                for dy in (-1, 0, 1):
                    for dx in (-1, 0, 1):
                        taps[t, k, dy + OY[i] + 2, dx + OX[i] + 2] += Wc[
                            t, k, i, dy + 1, dx + 1
                        ]
    return taps


def _build_bmats(Wc):
    """Dense lhsT matrices, returned as array [NB,128,128] f32 plus an index fn.

    Layout per (t,k): 5 generic B_Dx, then 3 top-variant (Dx=-1,0,1), then
    3 bottom-variant, then left corr, right corr = 13 matrices.
    B[in_row, out_row] = tap[in-out, Dx].
    """
    taps = _composite_taps(Wc)
    mats = []
    index = {}

    def band(vals_by_dy):
        B = np.zeros((128, 128), np.float64)
        for dy, v in vals_by_dy.items():
            B += v * np.eye(128, k=-dy)
        return B

    for t in range(T_ITERS):
        for k in range(KCH):
            per_dx = {}
            for Dx in (-2, -1, 0, 1, 2):
                per_dx[Dx] = band(
                    {Dy: taps[t, k, Dy + 2, Dx + 2] for Dy in range(-2, 3)}
                )
            for Dx in DXS:
                index[(t, k, Dx, "mid")] = len(mats)
                mats.append(per_dx[Dx])
            for Dx in (-1, 0, 1):
                Btop = per_dx[Dx].copy()
                # image row 0 = partition CORE_LO of tile 0: remove south-ch dy=-1
                Btop[CORE_LO, CORE_LO] -= Wc[t, k, 1, 0, Dx + 1]
                index[(t, k, Dx, "top")] = len(mats)
                mats.append(Btop)
            for Dx in (-1, 0, 1):
                Bbot = per_dx[Dx].copy()
                # image row 511 = partition 53 of tile 4: remove north-ch dy=+1
                p = CORE_LO + (H - 1) - TSTART[4] - CORE_LO  # = 53
                Bbot[p, p] -= Wc[t, k, 0, 2, Dx + 1]
                index[(t, k, Dx, "bot")] = len(mats)
                mats.append(Bbot)
            # column-edge corrections (vertical 3-tap bands)
            Bl = band({dy: -Wc[t, k, 3, dy + 1, 0] for dy in (-1, 0, 1)})
            index[(t, k, "corrL")] = len(mats)
            mats.append(Bl)
            Br = band({dy: -Wc[t, k, 2, dy + 1, 2] for dy in (-1, 0, 1)})
            index[(t, k, "corrR")] = len(mats)
            mats.append(Br)
    arr = np.stack(mats).astype(np.float32)
    return arr, index


_NB = T_ITERS * KCH * 13


def _build_masks():
    """Per-tile update masks [128, NTILES]: -C_UPD at real image rows, 0 at pad."""
    m = np.full((128, NTILES), -C_UPD, np.float32)
    for j in range(NTILES):
        r0 = TSTART[j]
        plo = max(0, -r0)
        phi = min(128, H - r0)
        m[0:plo, j] = 0.0
        m[phi:128, j] = 0.0
    return m


def _build_nc(bvals, bindex):
    nc = bacc.Bacc(None, target_bir_lowering=False)
    xs = nc.declare_dram_parameter("xs", [IMGS, H, W_IMG], F32, isOutput=False)
    bmw = _NB * 128 + (0 if _MM_DTYPE == "bf16" else NTILES)
    if _PAD_BMAT:
        bmw = (bmw + 127) // 128 * 128
    bdt = BF16 if _MM_DTYPE == "bf16" else F32
    bm = nc.declare_dram_parameter("bmat", [128, bmw], bdt, isOutput=False)
    if _MM_DTYPE == "bf16":
        aux = nc.declare_dram_parameter("aux", [128, 16], F32, isOutput=False)
    yo = nc.declare_dram_parameter("out", [IMGS, H, W_IMG], F32, isOutput=True)

    with TileContext(nc) as tc:
        with (
            tc.tile_pool(name="wts", bufs=1) as wp,
            tc.tile_pool(name="xdata", bufs=1) as xp,
            tc.tile_pool(name="work", bufs=2) as sp,
            tc.tile_pool(name="ps", bufs=_PSUM_BUFS, space="PSUM") as pp,
        ):
            bmt = wp.tile([128, bmw], bdt, tag="bmt")
            nc.sync.dma_start(out=bmt[:], in_=bm[:])
            if _MM_DTYPE == "bf16":
                auxt = wp.tile([128, 16], F32, tag="auxt")
                nc.sync.dma_start(out=auxt[:], in_=aux[:])

            if _SIMPLE_BIAS:
                bias_tiles = {}
                for t in range(T_ITERS):
                    for k in range(KCH):
                        bb = wp.tile([128, 1], F32, tag=f"bias{t}_{k}")
                        nc.vector.memset(bb[:], float(bvals[t, k]))
                        bias_tiles[t, k] = bb
            else:
                bias_t = wp.tile([128, T_ITERS * KCH], F32, tag="bias")
                for t in range(T_ITERS):
                    for k in range(KCH):
                        nc.vector.memset(
                            bias_t[:, t * KCH + k : t * KCH + k + 1], float(bvals[t, k])
                        )

            def bmat(idx):
                ap = bmt[:, idx * 128 : (idx + 1) * 128]
                if _MM_DTYPE == "f32r":
                    ap = ap.bitcast(F32R)
                return ap

            def mm_rhs(ap):
                if _MM_DTYPE == "f32r":
                    return ap.bitcast(F32R)
                return ap

            tset = _TILE_SET if _TILE_SET is not None else list(range(NTILES))
            xt = {}
            for im in range(IMGS):
                for j in tset:
                    tile = xp.tile([128, W_IMG], F32, tag=f"x{im}_{j}")
                    xt[im, j] = tile
                    r0 = TSTART[j]
                    plo = max(0, -r0)
                    phi = min(128, H - r0)
                    if plo > 0 or phi < 128:
                        nc.vector.memset(tile[:], 0.0)
                    nc.sync.dma_start(
                        out=tile[plo:phi, :], in_=xs[im, r0 + plo : r0 + phi, :]
                    )

            for it in range(T_ITERS):
                for im in range(IMGS):
                    for j in tset:
                        x_t = xt[im, j]
                        cls = "top" if j == 0 else ("bot" if j == NTILES - 1 else "mid")
                        if _MM_DTYPE == "bf16":
                            xmm = sp.tile([128, W_IMG], BF16, tag="xb")
                            nc.scalar.copy(xmm[:], x_t[:])
                        else:
                            xmm = x_t
                        d = pp.tile([128, KCH * W_IMG], F32, tag="d")
                        for k in range(KCH):
                            base = k * W_IMG
                            for Dx in DXS:
                                key = (
                                    (it, k, Dx, cls)
                                    if (it, k, Dx, cls) in bindex
                                    else (it, k, Dx, "mid")
                                )
                                ocl = max(0, -Dx)
                                och = W_IMG - max(0, Dx)
                                nc.tensor.matmul(
                                    d[:, base + ocl : base + och],
                                    bmat(bindex[key]),
                                    mm_rhs(xmm[:, ocl + Dx : och + Dx]),
                                    start=(Dx == 0),
                                    stop=False,
                                )
                            if not _SKIP_CORR:
                                nc.tensor.matmul(
                                    d[:, base : base + 1],
                                    bmat(bindex[(it, k, "corrL")]),
                                    mm_rhs(xmm[:, 0:1]),
                                    start=False,
                                    stop=False,
                                )
                                nc.tensor.matmul(
                                    d[:, base + W_IMG - 1 : base + W_IMG],
                                    bmat(bindex[(it, k, "corrR")]),
                                    mm_rhs(xmm[:, W_IMG - 1 : W_IMG]),
                                    start=False,
                                    stop=True,
                                )
                        e = sp.tile([128, KCH * W_IMG], F32, tag="e")
                        for k in range(KCH):
                            base = k * W_IMG
                            nc.scalar.activation(
                                e[:, base : base + W_IMG],
                                d[:, base : base + W_IMG],
                                AF.Derivative_Erf,
                                bias=(bias_tiles[it, k][:, 0:1] if _SIMPLE_BIAS
                                      else bias_t[:, it * KCH + k : it * KCH + k + 1]),
                                scale=1.0,
                            )
                        g = sp.tile([128, KCH * W_IMG], F32, tag="g")
                        for k in range(KCH):
                            base = k * W_IMG
                            nc.vector.scalar_tensor_tensor(
                                out=g[:, base : base + W_IMG],
                                in0=d[:, base : base + W_IMG],
                                scalar=float(bvals[it, k]),
                                in1=e[:, base : base + W_IMG],
                                op0=ALU.add,
                                op1=ALU.mult,
                            )
                        s01 = sp.tile([128, W_IMG], F32, tag="s01")
                        s23 = sp.tile([128, W_IMG], F32, tag="s23")
                        stot = sp.tile([128, W_IMG], F32, tag="stot")
                        nc.gpsimd.tensor_tensor(
                            out=s01[:], in0=g[:, 0:512], in1=g[:, 512:1024], op=ALU.add
                        )
                        nc.gpsimd.tensor_tensor(
                            out=s23[:], in0=g[:, 1024:1536], in1=g[:, 1536:2048],
                            op=ALU.add,
                        )
                        nc.gpsimd.tensor_tensor(
                            out=stot[:], in0=s01[:], in1=s23[:], op=ALU.add
                        )
                        if _MM_DTYPE == "bf16":
                            mask_ap = auxt[:, j : j + 1] if _MASK_AP else -C_UPD
                        else:
                            mask_ap = (
                                bmt[:, _NB * 128 + j : _NB * 128 + j + 1]
                                if _MASK_AP
                                else -C_UPD
                            )
                        if _INPLACE_UPD:
                            nc.vector.scalar_tensor_tensor(
                                out=x_t[:],
                                in0=stot[:],
                                scalar=mask_ap,
                                in1=x_t[:],
                                op0=ALU.mult,
                                op1=ALU.add,
                            )
                        else:
                            x_new = xp.tile([128, W_IMG], F32, tag=f"xn{im}_{j}_{it}")
                            nc.vector.scalar_tensor_tensor(
                                out=x_new[:],
                                in0=stot[:],
                                scalar=mask_ap,
                                in1=x_t[:],
                                op0=ALU.mult,
                                op1=ALU.add,
                            )
                            xt[im, j] = x_new

            for im in range(IMGS):
                for j in tset:
                    rows = CORE_ROWS[j]
                    nc.sync.dma_start(
                        out=yo[im, 116 * j : 116 * j + rows, :],
                        in_=xt[im, j][CORE_LO : CORE_LO + rows, :],
                    )
    nc.compile()
    return nc


_CACHE = {}


def _get_program(Wc, bc):
    key = (Wc.tobytes(), bc.tobytes())
    if key not in _CACHE:
        barr, bindex = _build_bmats(Wc.astype(np.float64))
        # SBUF layout [p, n*128+m]
        if _MM_DTYPE == "bf16":
            parts = [barr.transpose(1, 0, 2).reshape(128, _NB * 128)]
            w0 = _NB * 128
        else:
            parts = [barr.transpose(1, 0, 2).reshape(128, _NB * 128), _build_masks()]
            w0 = _NB * 128 + NTILES
        if _PAD_BMAT:
            wpad = (w0 + 127) // 128 * 128 - w0
            if wpad:
                parts.append(np.zeros((128, wpad), np.float32))
        bflat = np.ascontiguousarray(np.concatenate(parts, axis=1), dtype=np.float32)
        if _MM_DTYPE == "bf16":
            import ml_dtypes

            bflat = bflat.astype(ml_dtypes.bfloat16)
        nc = _build_nc(bc.astype(np.float64), bindex)
        _CACHE[key] = (nc, bflat)
    return _CACHE[key]


def _install_trace_shim():
    """The agent image lacks antenv.axon_hooks; rebuild the NTFF hook from
    trn_boot's ctypes recipe and skip the artifact upload."""
    import types

    if "antenv.axon_hooks" in sys.modules:
        return
    try:
        from trn_agent_boot.trn_boot import _ntff_profile_via_ctypes

        hook = _ntff_profile_via_ctypes("/opt/axon/libaxon_pjrt.so")
    except Exception:
        hook = None
    mod = types.ModuleType("antenv.axon_hooks")
    mod.get_axon_ntff_profile_hook = lambda: hook
    mod.set_axon_ntff_profile_hook = lambda h: None
    sys.modules["antenv.axon_hooks"] = mod
    import concourse.bass_utils as bu

    bu.upload_artifacts = lambda d: "local://skipped"


def kernel(x, W, b, _trace=False, _tracedir=None):
    x = np.asarray(x)
    W = np.asarray(W)
    b = np.asarray(b)
    nc, bflat = _get_program(W, b)
    in_maps = []
    for c in range(NCORES):
        shard = np.ascontiguousarray(x[c * IMGS : (c + 1) * IMGS, 0]).astype(np.float32)
        im_map = {"xs": shard, "bmat": bflat}
        if _MM_DTYPE == "bf16":
            am = np.zeros((128, 16), np.float32)
            am[:, :NTILES] = _build_masks()
            im_map["aux"] = am
        in_maps.append(im_map)
    kw = {}
    if _trace:
        _install_trace_shim()
        kw = {"trace": True, "tmpdir": _tracedir}
    res = run_bass_kernel_spmd(nc, in_maps, list(range(NCORES)), **kw)
    out = np.concatenate([res.results[c]["out"] for c in range(NCORES)], axis=0)
    out = out[:, None].astype(x.dtype)
    kernel._last = res
    return out
